# revision 25
# baseline (speedup 1.0000x reference)
"""Low-dim-QK multi-head attention TRN2 Bass kernel (8 NeuronCores).

Reference computation (all fp32):
  Ql = (Q @ Wq.T + bq)  -> (B, TQ, 256) -> heads (B, 8, TQ, 32)
  Kl = (K @ Wk.T + bk)  -> heads (B, 8, TK, 32)
  S  = Ql @ Kl.T / sqrt(32),  masked by key_padding_mask (-inf)
  A  = softmax(S, axis=-1)
  out = concat_h(A_h @ V) @ Wo.T + bo        # V shared across heads

Sharding: (batch, q-chunk) across 8 cores: core c handles batch c//4,
query rows [(c%4)*512, (c%4)*512+512).  No inter-core communication.

Key compaction: masked keys contribute exactly nothing to softmax attention
(their scores are -inf => weight 0), so the host gathers only the unmasked
K/V rows per batch and pads to a k-tile multiple; padded slots are killed
with an exp bias of -60.  With the ~50% random mask this halves all
key-dimension work.

On-chip layout (everything transposed so no transposes are ever needed):
  qlT/klT: (r=head*32+hd on partitions, tokens free)  from host-transposed
           Q.T/K.T and Wq.T/Wk.T via out = Wq @ Q.T-style matmuls.
  S.T     per (head, k-tile): matmul(lhsT=klT[32,128], rhs=qlT[32,512]).
  P.T     = exp(S.T * scale + padbias[k]) on ScalarE (bias is per-partition
           = per-key, which implements padding-kill for free).
  D       = softmax denominators via ones-column matmuls (PSUM fp32 accum).
  X.T     per (head, d-tile): matmul(lhsT=V[k,d], rhs=P.T) accumulated over
           k-tiles, then normalized by broadcast 1/D on VectorE.
  out     = matmul(lhsT=X.T[c,q], rhs=Wo.T[c,o]) accumulated per head in
           PSUM, then accumulated across heads in SBUF fp32.

Matmuls run in float32r (full PE rate, ~1e-4 relative precision) with fp32
PSUM accumulation.
"""

import math

import numpy as np

import concourse.bass as bass
import concourse.mybir as mybir
import concourse.tile as tile
from concourse import bacc
from concourse.bass_utils import run_bass_kernel_spmd

F32 = mybir.dt.float32
F32R = mybir.dt.float32r
AF = mybir.ActivationFunctionType

B = 2
TQ = 2048
TK = 2048
D = 1024          # model dim of Q/K/V inputs
R = 256           # QK_DIM
H = 8
HD = 32           # head dim of Ql/Kl
DV = 1024         # V dim (shared across heads)
O = 1024          # output dim
NCORES = 8
TQC = TQ * B // NCORES   # 512 query rows per core
DT = D // 128            # 8 contraction tiles for projections
VT = DV // 128           # 8 d-tiles of V / c-tiles per head
QS = TQC // 128          # 4 query sub-tiles (M=128 each)
SCALE = 1.0 / float(np.sqrt(HD))
MASK_BIAS = -60.0


def _kproj_chunks(tkp):
    """Split the padded key count into matmul-N chunks (>=256 keeps f32r at
    full rate; the tail chunk may be smaller)."""
    if tkp % 384 == 0:
        return [384] * (tkp // 384)
    return [512] * (tkp // 512) + ([tkp % 512] if tkp % 512 else [])


def _body(nc, t, pools, tkp, n_heads=H):
    qT, kT, v, wqT, wkT, woT, bq, bk, bo, mb, ones_col_d, ones_row_d, out = t
    (const, wpool, pt_pool, xt_pool, wo_pool, smalls,
     ps_s, ps_d, ps_o, ps_p) = pools
    kt_tiles = tkp // 128

    # ---- persistent SBUF tensors -------------------------------------
    v_sb = const.tile([128, kt_tiles, DV], F32R, tag="v")   # V[k,d]; part = k%128
    klT_sb = const.tile([128, 2, tkp], F32R, tag="klT")     # part = r%128, g = r//128
    qlT_sb = const.tile([128, 2, TQC], F32R, tag="qlT")
    mb_sb = const.tile([128, kt_tiles], F32, tag="mb")      # pad-kill bias per key
    bq_sb = const.tile([128, 2], F32, tag="bq")
    bk_sb = const.tile([128, 2], F32, tag="bk")
    rbo_sb = const.tile([128, O], F32, tag="rbo")           # bo replicated over parts
    ones_col = const.tile([128, 1], F32R, tag="onc")
    ones_row = const.tile([1, 128], F32R, tag="onr")
    out_acc = const.tile([128, QS, O], F32, tag="oacc")     # part = q%128

    # ---- phase A: Q/K low-dim projections ----------------------------
    # DMA emission order matters: projection weights + streams first (they
    # gate the first matmuls), bulk V / out-proj constants later.
    wq_sb = wpool.tile([128, DT, R], F32R, tag="wq")
    wk_sb = wpool.tile([128, DT, R], F32R, tag="wk")
    wq_r = wqT.ap().rearrange("(dt p) r -> p dt r", p=128)
    nc.sync.dma_start(out=wq_sb[:, 0:2, :], in_=wq_r[:, 0:2, :])

    # Ql.T = Wq @ Q.T: contraction-chunk streaming through the pt pool,
    # both r-groups (g) accumulate in parallel PSUM banks.
    ql_ps = [ps_s.tile([128, TQC], F32, tag="st", name=f"qlps{g}") for g in range(2)]
    for dt_ in range(DT):
        qc = pt_pool.tile([128, TQC], F32R, tag="pt")
        nc.sync.dma_start(
            out=qc,
            in_=qT.ap()[dt_ * 128 : (dt_ + 1) * 128, :],
        )
        if dt_ == 0:
            nc.sync.dma_start(out=wq_sb[:, 2:, :], in_=wq_r[:, 2:, :])
        if dt_ == 3:
            nc.sync.dma_start(
                out=wk_sb, in_=wkT.ap().rearrange("(dt p) r -> p dt r", p=128)
            )
            nc.sync.dma_start(out=bq_sb, in_=bq.ap().rearrange("(g p) -> p g", p=128))
        for g in range(2):
            nc.tensor.matmul(
                ql_ps[g],
                lhsT=wq_sb[:, dt_, g * 128 : (g + 1) * 128],
                rhs=qc,
                start=(dt_ == 0),
                stop=(dt_ == DT - 1),
            )
    for g in range(2):
        nc.scalar.activation(
            out=qlT_sb[:, g, :], in_=ql_ps[g], func=AF.Identity,
            bias=bq_sb[:, g : g + 1], scale=1.0,
        )

    # Kl.T in token chunks
    k0 = 0
    for nt, csz in enumerate(_kproj_chunks(tkp)):
        kl_ps = [
            ps_s.tile([128, csz], F32, tag="st", name=f"klps{nt}_{g}")
            for g in range(2)
        ]
        for dt_ in range(DT):
            kc = pt_pool.tile([128, csz], F32R, tag="pt", name=f"kc{nt}_{dt_}")
            nc.sync.dma_start(
                out=kc,
                in_=kT.ap()[dt_ * 128 : (dt_ + 1) * 128, k0 : k0 + csz],
            )
            if nt == 0 and dt_ == 0:
                # remaining small constants behind the first K chunk
                nc.sync.dma_start(
                    out=bk_sb, in_=bk.ap().rearrange("(g p) -> p g", p=128)
                )
                nc.sync.dma_start(
                    out=mb_sb, in_=mb.ap().rearrange("(kt p) -> p kt", p=128)
                )
                nc.sync.dma_start(out=ones_col, in_=ones_col_d.ap())
                nc.sync.dma_start(out=ones_row, in_=ones_row_d.ap())
            for g in range(2):
                nc.tensor.matmul(
                    kl_ps[g],
                    lhsT=wk_sb[:, dt_, g * 128 : (g + 1) * 128],
                    rhs=kc,
                    start=(dt_ == 0),
                    stop=(dt_ == DT - 1),
                )
        for g in range(2):
            nc.scalar.activation(
                out=klT_sb[:, g, k0 : k0 + csz], in_=kl_ps[g],
                func=AF.Identity, bias=bk_sb[:, g : g + 1], scale=1.0,
            )
        k0 += csz

    # bulk loads needed from the attnV phase onwards, queued behind the
    # projection streams on purpose
    vstep = 3 if kt_tiles % 3 == 0 else 4
    for vc in range(0, kt_tiles, vstep):
        hi = min(vc + vstep, kt_tiles)
        nc.sync.dma_start(
            out=v_sb[:, vc:hi, :],
            in_=v.ap()[vc * 128 : hi * 128, :].rearrange(
                "(kt p) d -> p kt d", p=128
            ),
        )
    bo_bcast = bass.AP(tensor=bo.ap().tensor, offset=0, ap=[[0, 128], [1, O]])
    nc.sync.dma_start(out=rbo_sb, in_=bo_bcast)

    # ---- phase B: per-head attention + fused out-projection ----------
    def emit_scores(h):
        """scores.T -> exp -> P.T per k-tile, with the softmax denominator
        and the first d-tile of attnV interleaved so PE paces with ScalarE."""
        g, pb = divmod(h, 4)
        pb *= 32
        pts = []
        d_ps = ps_d.tile([1, TQC], F32, tag="d", name=f"dps{h}")
        o_ps0 = ps_o.tile([128, TQC], F32, tag="o", name=f"ops0_{h}")
        for kt_ in range(kt_tiles):
            st = ps_s.tile([128, TQC], F32, tag="st", name=f"st{h}_{kt_}")
            nc.tensor.matmul(
                st,
                lhsT=klT_sb[pb : pb + 32, g, kt_ * 128 : (kt_ + 1) * 128],
                rhs=qlT_sb[pb : pb + 32, g, :],
                start=True,
                stop=True,
                tile_position=(pb, 0),
            )
            pt = pt_pool.tile([128, TQC], F32R, tag="pt", name=f"pt{h}_{kt_}")
            nc.scalar.activation(
                out=pt, in_=st, func=AF.Exp,
                bias=mb_sb[:, kt_ : kt_ + 1], scale=SCALE,
            )
            pts.append(pt)
            nc.tensor.matmul(
                d_ps, lhsT=ones_col, rhs=pt,
                start=(kt_ == 0), stop=(kt_ == kt_tiles - 1),
            )
            nc.tensor.matmul(
                o_ps0, lhsT=v_sb[:, kt_, 0:128], rhs=pt,
                start=(kt_ == 0), stop=(kt_ == kt_tiles - 1),
            )
        return pts, d_ps, o_ps0

    def emit_attnv(h, pts, d_ps, o_ps0):
        """normalizer broadcast + remaining attnV d-tiles -> normalized X.T"""
        d_sb = smalls.tile([1, TQC], F32R, tag="dsb", name=f"dsb{h}")
        nc.vector.reciprocal(out=d_sb, in_=d_ps)
        rep_ps = ps_s.tile([128, TQC], F32, tag="st", name=f"repps{h}")
        nc.tensor.matmul(rep_ps, lhsT=ones_row, rhs=d_sb, start=True, stop=True)
        rep_sb = smalls.tile([128, TQC], F32, tag="rep", name=f"rep{h}")
        nc.vector.tensor_copy(out=rep_sb, in_=rep_ps)

        xts = []
        for dt_ in range(VT):
            if dt_ == 0:
                o_ps = o_ps0
            else:
                o_ps = ps_o.tile([128, TQC], F32, tag="o", name=f"ops{h}_{dt_}")
                for kt_ in range(len(pts)):
                    nc.tensor.matmul(
                        o_ps,
                        lhsT=v_sb[:, kt_, dt_ * 128 : (dt_ + 1) * 128],
                        rhs=pts[kt_],
                        start=(kt_ == 0),
                        stop=(kt_ == len(pts) - 1),
                    )
            xt = xt_pool.tile([128, TQC], F32R, tag="xt", name=f"xt{h}_{dt_}")
            nc.vector.tensor_mul(xt, o_ps, rep_sb)
            xts.append(xt)
        return xts

    def emit_outproj(h, xts, out_r=None):
        """out[q,o] += sum_{c in head h} X.T[c,q].T @ Wo.T[c,o].

        For the last head (out_r given) iterate qs-major with all Wo chunks
        preloaded, storing each finished q sub-tile immediately so the final
        DMA overlaps the remaining matmuls."""

        def load_wo(oj, cp):
            wo_sb = wo_pool.tile(
                [128, 2, 512], F32R, tag="wo", name=f"wo{h}_{oj}_{cp}"
            )
            nc.sync.dma_start(
                out=wo_sb,
                in_=woT.ap()[
                    h * DV + cp * 256 : h * DV + (cp + 1) * 256,
                    oj * 512 : (oj + 1) * 512,
                ].rearrange("(c p) o -> p c o", p=128),
            )
            return wo_sb

        def op_group(oj, qs_, wos):
            op_ps = ps_p.tile(
                [128, 512], F32, tag="op", name=f"opps{h}_{oj}_{qs_}"
            )
            for ct in range(VT):
                nc.tensor.matmul(
                    op_ps,
                    lhsT=xts[ct][:, qs_ * 128 : (qs_ + 1) * 128],
                    rhs=wos[ct // 2][:, ct % 2, :],
                    start=(ct == 0),
                    stop=(ct == VT - 1),
                )
            acc = out_acc[:, qs_, oj * 512 : (oj + 1) * 512]
            if h == 0:
                nc.vector.tensor_add(
                    acc, op_ps, rbo_sb[:, oj * 512 : (oj + 1) * 512]
                )
            else:
                nc.vector.tensor_add(acc, op_ps, acc)

        if out_r is None:
            for oj in range(2):
                wos = [load_wo(oj, cp) for cp in range(VT // 2)]
                for qs_ in range(QS):
                    op_group(oj, qs_, wos)
        else:
            wos2 = [[load_wo(oj, cp) for cp in range(VT // 2)] for oj in range(2)]
            for qs_ in range(QS):
                for oj in range(2):
                    op_group(oj, qs_, wos2[oj])
                nc.sync.dma_start(out=out_r[:, qs_, :], in_=out_acc[:, qs_, :])

    # Software pipeline across heads: head h+1's scores are emitted before
    # head h's out-projection, so ScalarE computes h+1's exps while the PE
    # is busy with h's out-projection matmuls.
    out_r = out.ap().rearrange("(qs p) o -> p qs o", p=128)
    fuse_store = tkp <= 1536
    sc = emit_scores(0)
    for h in range(n_heads):
        xts = emit_attnv(h, *sc)
        if h + 1 < n_heads:
            sc = emit_scores(h + 1)
        last = h == n_heads - 1
        emit_outproj(h, xts, out_r=(out_r if last and fuse_store else None))
    if not fuse_store:
        for qs_ in range(QS):
            nc.sync.dma_start(out=out_r[:, qs_, :], in_=out_acc[:, qs_, :])


def _build_kernel(nc, t, tkp, loop_n=None, n_heads=H):
    tc = tile.TileContext(nc)
    with tc, nc.allow_low_precision(
        reason="float32r matmul operands; all accumulation is fp32 in PSUM/SBUF"
    ):
        pools = (
            tc.alloc_tile_pool(name="const", bufs=1),
            tc.alloc_tile_pool(name="wpool", bufs=1),
            tc.alloc_tile_pool(name="pt", bufs=(2 * (tkp // 128) + 2 if tkp <= 1536 else tkp // 128)),
            tc.alloc_tile_pool(name="xt", bufs=9),
            tc.alloc_tile_pool(name="wo", bufs=9 if tkp <= 1536 else 5),
            tc.alloc_tile_pool(name="smalls", bufs=2),
            tc.alloc_tile_pool(name="ps_s", bufs=2, space="PSUM"),
            tc.alloc_tile_pool(name="ps_d", bufs=2, space="PSUM"),
            tc.alloc_tile_pool(name="ps_o", bufs=2, space="PSUM"),
            tc.alloc_tile_pool(name="ps_p", bufs=2, space="PSUM"),
        )
        if loop_n is None:
            _body(nc, t, pools, tkp, n_heads=n_heads)
        else:
            with tc.For_i(0, loop_n, 1):
                _body(nc, t, pools, tkp, n_heads=n_heads)
        for p in reversed(pools):
            p.release()
    nc.compile()


def _declare_io(nc, tkp):
    qT = nc.dram_tensor("qT", (D, TQC), F32R, kind="ExternalInput")
    kT = nc.dram_tensor("kT", (D, tkp), F32R, kind="ExternalInput")
    v = nc.dram_tensor("v", (tkp, DV), F32R, kind="ExternalInput")
    wqT = nc.dram_tensor("wqT", (D, R), F32R, kind="ExternalInput")
    wkT = nc.dram_tensor("wkT", (D, R), F32R, kind="ExternalInput")
    woT = nc.dram_tensor("woT", (H * DV, O), F32R, kind="ExternalInput")
    bq = nc.dram_tensor("bq", (R,), F32, kind="ExternalInput")
    bk = nc.dram_tensor("bk", (R,), F32, kind="ExternalInput")
    bo = nc.dram_tensor("bo", (O,), F32, kind="ExternalInput")
    mb = nc.dram_tensor("mb", (tkp,), F32, kind="ExternalInput")
    ones_col_d = nc.dram_tensor("ones_col_d", (128, 1), F32R, kind="ExternalInput")
    ones_row_d = nc.dram_tensor("ones_row_d", (1, 128), F32R, kind="ExternalInput")
    out = nc.dram_tensor("out", (TQC, O), F32, kind="ExternalOutput")
    return (qT, kT, v, wqT, wkT, woT, bq, bk, bo, mb, ones_col_d, ones_row_d, out)


def build_nc(tkp=1024, loop_n=None, n_heads=H):
    nc = bacc.Bacc("TRN2", target_bir_lowering=False, debug=False,
                   num_devices=NCORES)
    t = _declare_io(nc, tkp)
    _build_kernel(nc, t, tkp, loop_n=loop_n, n_heads=n_heads)
    return nc


_NC_CACHE = {}


def _get_nc(tkp):
    if tkp not in _NC_CACHE:
        _NC_CACHE[tkp] = build_nc(tkp)
    return _NC_CACHE[tkp]


def _prep_in_maps(Q, K, V, Wq, bq, Wk, bk, Wo, bo, key_padding_mask):
    Q = np.asarray(Q, dtype=np.float32)
    K = np.asarray(K, dtype=np.float32)
    V = np.asarray(V, dtype=np.float32)
    Wq = np.asarray(Wq, dtype=np.float32)
    Wk = np.asarray(Wk, dtype=np.float32)
    Wo = np.asarray(Wo, dtype=np.float32)
    bq = np.ascontiguousarray(np.asarray(bq, dtype=np.float32))
    bk = np.ascontiguousarray(np.asarray(bk, dtype=np.float32))
    bo = np.ascontiguousarray(np.asarray(bo, dtype=np.float32))
    mask = np.asarray(key_padding_mask).astype(bool)

    # key compaction: keep only unmasked keys, pad to a 128 multiple
    keep = [np.flatnonzero(~mask[b]) for b in range(B)]
    tkp = max(128, ((max(len(ix) for ix in keep) + 127) // 128) * 128)

    wqT = np.ascontiguousarray(Wq.T)
    wkT = np.ascontiguousarray(Wk.T)
    woT = np.ascontiguousarray(Wo.T)
    ones_col = np.ones((128, 1), np.float32)
    ones_row = np.ones((1, 128), np.float32)
    kT, vb, mb = [], [], []
    for b in range(B):
        ix = keep[b]
        kt = np.zeros((D, tkp), np.float32)
        kt[:, : len(ix)] = K[b].T[:, ix]
        kT.append(kt)
        vv = np.zeros((tkp, DV), np.float32)
        vv[: len(ix)] = V[b][ix]
        vb.append(vv)
        m = np.full(tkp, np.float32(MASK_BIAS), np.float32)
        m[: len(ix)] = 0.0
        mb.append(m)

    in_maps = []
    for c in range(NCORES):
        b, chunk = divmod(c, NCORES // B)
        q0 = chunk * TQC
        in_maps.append(
            {
                "qT": np.ascontiguousarray(Q[b, q0 : q0 + TQC, :].T),
                "kT": kT[b],
                "v": vb[b],
                "wqT": wqT,
                "wkT": wkT,
                "woT": woT,
                "bq": bq,
                "bk": bk,
                "bo": bo,
                "mb": mb[b],
                "ones_col_d": ones_col,
                "ones_row_d": ones_row,
            }
        )
    return tkp, in_maps


def run(inputs: dict, **spmd_kwargs):
    """Build (cached), run on 8 cores, return (full_output, BassKernelResults)."""
    tkp, in_maps = _prep_in_maps(**inputs)
    nc = _get_nc(tkp)
    res = run_bass_kernel_spmd(nc, in_maps, core_ids=list(range(NCORES)),
                               **spmd_kwargs)
    out = np.empty((B, TQ, O), np.float32)
    for c in range(NCORES):
        b, chunk = divmod(c, NCORES // B)
        q0 = chunk * TQC
        out[b, q0 : q0 + TQC, :] = res.results[c]["out"]
    return out, res


def kernel(**inputs) -> np.ndarray:
    out, _ = run(inputs)
    return out


# revision 28
# speedup vs baseline: 1.0437x; 1.0437x over previous
"""Low-dim-QK multi-head attention TRN2 Bass kernel (8 NeuronCores).

Reference computation (all fp32):
  Ql = (Q @ Wq.T + bq)  -> (B, TQ, 256) -> heads (B, 8, TQ, 32)
  Kl = (K @ Wk.T + bk)  -> heads (B, 8, TK, 32)
  S  = Ql @ Kl.T / sqrt(32),  masked by key_padding_mask (-inf)
  A  = softmax(S, axis=-1)
  out = concat_h(A_h @ V) @ Wo.T + bo        # V shared across heads

Sharding: (batch, q-chunk) across 8 cores: core c handles batch c//4,
query rows [(c%4)*512, (c%4)*512+512).  No inter-core communication.

Key compaction: masked keys contribute exactly nothing to softmax attention
(their scores are -inf => weight 0), so the host gathers only the unmasked
K/V rows per batch and pads to a k-tile multiple; padded slots are killed
with an exp bias of -60.  With the ~50% random mask this halves all
key-dimension work.

On-chip layout (everything transposed so no transposes are ever needed):
  qlT/klT: (r=head*32+hd on partitions, tokens free)  from host-transposed
           Q.T/K.T and Wq.T/Wk.T via out = Wq @ Q.T-style matmuls.
  S.T     per (head, k-tile): matmul(lhsT=klT[32,128], rhs=qlT[32,512]).
  P.T     = exp(S.T * scale + padbias[k]) on ScalarE (bias is per-partition
           = per-key, which implements padding-kill for free).
  D       = softmax denominators via ones-column matmuls (PSUM fp32 accum).
  X.T     per (head, d-tile): matmul(lhsT=V[k,d], rhs=P.T) accumulated over
           k-tiles, then normalized by broadcast 1/D on VectorE.
  out     = matmul(lhsT=X.T[c,q], rhs=Wo.T[c,o]) accumulated per head in
           PSUM, then accumulated across heads in SBUF fp32.

Matmuls run in float32r (full PE rate, ~1e-4 relative precision) with fp32
PSUM accumulation.
"""

import math

import numpy as np

import concourse.bass as bass
import concourse.mybir as mybir
import concourse.tile as tile
from concourse import bacc
from concourse.bass_utils import run_bass_kernel_spmd

F32 = mybir.dt.float32
F32R = mybir.dt.float32r
AF = mybir.ActivationFunctionType

B = 2
TQ = 2048
TK = 2048
D = 1024          # model dim of Q/K/V inputs
R = 256           # QK_DIM
H = 8
HD = 32           # head dim of Ql/Kl
DV = 1024         # V dim (shared across heads)
O = 1024          # output dim
NCORES = 8
TQC = TQ * B // NCORES   # 512 query rows per core
DT = D // 128            # 8 contraction tiles for projections
VT = DV // 128           # 8 d-tiles of V / c-tiles per head
QS = TQC // 128          # 4 query sub-tiles (M=128 each)
SCALE = 1.0 / float(np.sqrt(HD))
MASK_BIAS = -60.0


def _kproj_chunks(tkp):
    """Split the padded key count into matmul-N chunks (>=256 keeps f32r at
    full rate; the tail chunk may be smaller)."""
    if tkp % 384 == 0:
        return [384] * (tkp // 384)
    return [512] * (tkp // 512) + ([tkp % 512] if tkp % 512 else [])


def _body(nc, t, pools, tkp, n_heads=H):
    qT, kT, v, wqT, wkT, woT, bq, bk, bo, mb, ones_col_d, ones_row_d, out = t
    (const, wpool, pt_pool, xt_pool, wo_pool, smalls,
     ps_s, ps_d, ps_o, ps_p) = pools
    kt_tiles = tkp // 128

    # ---- persistent SBUF tensors -------------------------------------
    v_sb = const.tile([128, kt_tiles, DV], F32R, tag="v")   # V[k,d]; part = k%128
    klT_sb = const.tile([128, 2, tkp], F32R, tag="klT")     # part = r%128, g = r//128
    qlT_sb = const.tile([128, 2, TQC], F32R, tag="qlT")
    mb_sb = const.tile([128, kt_tiles], F32, tag="mb")      # pad-kill bias per key
    bq_sb = const.tile([128, 2], F32, tag="bq")
    bk_sb = const.tile([128, 2], F32, tag="bk")
    rbo_sb = const.tile([128, O], F32, tag="rbo")           # bo replicated over parts
    ones_col = const.tile([128, 1], F32R, tag="onc")
    ones_row = const.tile([1, 128], F32R, tag="onr")
    out_acc = const.tile([128, QS, O], F32, tag="oacc")     # part = q%128

    # ---- phase A: Q/K low-dim projections ----------------------------
    # DMA emission order matters: projection weights + streams first (they
    # gate the first matmuls), bulk V / out-proj constants later.
    wq_sb = wpool.tile([128, DT, R], F32R, tag="wq")
    wk_sb = wpool.tile([128, DT, R], F32R, tag="wk")
    wq_r = wqT.ap().rearrange("(dt p) r -> p dt r", p=128)
    nc.sync.dma_start(out=wq_sb[:, 0:2, :], in_=wq_r[:, 0:2, :])

    # Ql.T = Wq @ Q.T: contraction-chunk streaming through the pt pool,
    # both r-groups (g) accumulate in parallel PSUM banks.
    ql_ps = [ps_s.tile([128, TQC], F32, tag="st", name=f"qlps{g}") for g in range(2)]
    for dt_ in range(DT):
        qc = pt_pool.tile([128, TQC], F32R, tag="pt")
        nc.sync.dma_start(
            out=qc,
            in_=qT.ap()[dt_ * 128 : (dt_ + 1) * 128, :],
        )
        if dt_ == 0:
            nc.sync.dma_start(out=wq_sb[:, 2:, :], in_=wq_r[:, 2:, :])
        if dt_ == 3:
            nc.sync.dma_start(
                out=wk_sb, in_=wkT.ap().rearrange("(dt p) r -> p dt r", p=128)
            )
            nc.sync.dma_start(out=bq_sb, in_=bq.ap().rearrange("(g p) -> p g", p=128))
        for g in range(2):
            nc.tensor.matmul(
                ql_ps[g],
                lhsT=wq_sb[:, dt_, g * 128 : (g + 1) * 128],
                rhs=qc,
                start=(dt_ == 0),
                stop=(dt_ == DT - 1),
            )
    for g in range(2):
        nc.scalar.activation(
            out=qlT_sb[:, g, :], in_=ql_ps[g], func=AF.Identity,
            bias=bq_sb[:, g : g + 1], scale=1.0,
        )

    # Kl.T in token chunks
    k0 = 0
    for nt, csz in enumerate(_kproj_chunks(tkp)):
        kl_ps = [
            ps_s.tile([128, csz], F32, tag="st", name=f"klps{nt}_{g}")
            for g in range(2)
        ]
        for dt_ in range(DT):
            kc = pt_pool.tile([128, csz], F32R, tag="pt", name=f"kc{nt}_{dt_}")
            nc.sync.dma_start(
                out=kc,
                in_=kT.ap()[dt_ * 128 : (dt_ + 1) * 128, k0 : k0 + csz],
            )
            if nt == 0 and dt_ == 0:
                # remaining small constants behind the first K chunk
                nc.sync.dma_start(
                    out=bk_sb, in_=bk.ap().rearrange("(g p) -> p g", p=128)
                )
                nc.sync.dma_start(
                    out=mb_sb, in_=mb.ap().rearrange("(kt p) -> p kt", p=128)
                )
                nc.sync.dma_start(out=ones_col, in_=ones_col_d.ap())
                nc.sync.dma_start(out=ones_row, in_=ones_row_d.ap())
            for g in range(2):
                nc.tensor.matmul(
                    kl_ps[g],
                    lhsT=wk_sb[:, dt_, g * 128 : (g + 1) * 128],
                    rhs=kc,
                    start=(dt_ == 0),
                    stop=(dt_ == DT - 1),
                )
        for g in range(2):
            nc.scalar.activation(
                out=klT_sb[:, g, k0 : k0 + csz], in_=kl_ps[g],
                func=AF.Identity, bias=bk_sb[:, g : g + 1], scale=1.0,
            )
        k0 += csz

    # bulk loads needed from the attnV phase onwards, queued behind the
    # projection streams on purpose
    vstep = 3 if kt_tiles % 3 == 0 else 4
    for vc in range(0, kt_tiles, vstep):
        hi = min(vc + vstep, kt_tiles)
        nc.sync.dma_start(
            out=v_sb[:, vc:hi, :],
            in_=v.ap()[vc * 128 : hi * 128, :].rearrange(
                "(kt p) d -> p kt d", p=128
            ),
        )
    bo_bcast = bass.AP(tensor=bo.ap().tensor, offset=0, ap=[[0, 128], [1, O]])
    nc.sync.dma_start(out=rbo_sb, in_=bo_bcast)

    # ---- phase B: per-head attention + fused out-projection ----------
    def emit_scores(h):
        """scores.T -> exp -> P.T per k-tile, with the softmax denominator
        and the first d-tile of attnV interleaved so PE paces with ScalarE."""
        g, pb = divmod(h, 4)
        pb *= 32
        pts = []
        d_ps = ps_d.tile([1, TQC], F32, tag="d", name=f"dps{h}")
        o_ps0 = ps_o.tile([128, TQC], F32, tag="o", name=f"ops0_{h}")
        for kt_ in range(kt_tiles):
            st = ps_s.tile([128, TQC], F32, tag="st", name=f"st{h}_{kt_}")
            nc.tensor.matmul(
                st,
                lhsT=klT_sb[pb : pb + 32, g, kt_ * 128 : (kt_ + 1) * 128],
                rhs=qlT_sb[pb : pb + 32, g, :],
                start=True,
                stop=True,
                tile_position=(pb, 0),
            )
            pt = pt_pool.tile([128, TQC], F32R, tag="pt", name=f"pt{h}_{kt_}")
            nc.scalar.activation(
                out=pt, in_=st, func=AF.Exp,
                bias=mb_sb[:, kt_ : kt_ + 1], scale=SCALE,
            )
            pts.append(pt)
            nc.tensor.matmul(
                d_ps, lhsT=ones_col, rhs=pt,
                start=(kt_ == 0), stop=(kt_ == kt_tiles - 1),
            )
            nc.tensor.matmul(
                o_ps0, lhsT=v_sb[:, kt_, 0:128], rhs=pt,
                start=(kt_ == 0), stop=(kt_ == kt_tiles - 1),
            )
        return pts, d_ps, o_ps0

    def emit_attnv(h, pts, d_ps, o_ps0):
        """normalizer broadcast + remaining attnV d-tiles -> normalized X.T"""
        d_sb = smalls.tile([1, TQC], F32R, tag="dsb", name=f"dsb{h}")
        nc.vector.reciprocal(out=d_sb, in_=d_ps)
        rep_ps = ps_s.tile([128, TQC], F32, tag="st", name=f"repps{h}")
        nc.tensor.matmul(rep_ps, lhsT=ones_row, rhs=d_sb, start=True, stop=True)
        rep_sb = smalls.tile([128, TQC], F32, tag="rep", name=f"rep{h}")
        nc.vector.tensor_copy(out=rep_sb, in_=rep_ps)

        xts = []
        for dt_ in range(VT):
            if dt_ == 0:
                o_ps = o_ps0
            else:
                o_ps = ps_o.tile([128, TQC], F32, tag="o", name=f"ops{h}_{dt_}")
                for kt_ in range(len(pts)):
                    nc.tensor.matmul(
                        o_ps,
                        lhsT=v_sb[:, kt_, dt_ * 128 : (dt_ + 1) * 128],
                        rhs=pts[kt_],
                        start=(kt_ == 0),
                        stop=(kt_ == len(pts) - 1),
                    )
            xt = xt_pool.tile([128, TQC], F32R, tag="xt", name=f"xt{h}_{dt_}")
            nc.vector.tensor_mul(xt, o_ps, rep_sb)
            xts.append(xt)
        return xts

    def emit_outproj(h, xts, out_r=None):
        """out[q,o] += sum_{c in head h} X.T[c,q].T @ Wo.T[c,o].

        For the last head (out_r given) iterate qs-major with all Wo chunks
        preloaded, storing each finished q sub-tile immediately so the final
        DMA overlaps the remaining matmuls."""

        def load_wo(oj, cp):
            wo_sb = wo_pool.tile(
                [128, 2, 512], F32R, tag="wo", name=f"wo{h}_{oj}_{cp}"
            )
            nc.sync.dma_start(
                out=wo_sb,
                in_=woT.ap()[
                    h * DV + cp * 256 : h * DV + (cp + 1) * 256,
                    oj * 512 : (oj + 1) * 512,
                ].rearrange("(c p) o -> p c o", p=128),
            )
            return wo_sb

        def op_group(oj, qs_, wos):
            op_ps = ps_p.tile(
                [128, 512], F32, tag="op", name=f"opps{h}_{oj}_{qs_}"
            )
            for ct in range(VT):
                nc.tensor.matmul(
                    op_ps,
                    lhsT=xts[ct][:, qs_ * 128 : (qs_ + 1) * 128],
                    rhs=wos[ct // 2][:, ct % 2, :],
                    start=(ct == 0),
                    stop=(ct == VT - 1),
                )
            acc = out_acc[:, qs_, oj * 512 : (oj + 1) * 512]
            if h == 0:
                nc.vector.tensor_add(
                    acc, op_ps, rbo_sb[:, oj * 512 : (oj + 1) * 512]
                )
            else:
                nc.vector.tensor_add(acc, op_ps, acc)

        if out_r is None:
            for oj in range(2):
                wos = [load_wo(oj, cp) for cp in range(VT // 2)]
                for qs_ in range(QS):
                    op_group(oj, qs_, wos)
        else:
            wos2 = [[load_wo(oj, cp) for cp in range(VT // 2)] for oj in range(2)]
            for qs_ in range(QS):
                for oj in range(2):
                    op_group(oj, qs_, wos2[oj])
                    # store each finished 256KB half immediately so the
                    # final store only covers the last op-group's output
                    nc.sync.dma_start(
                        out=out_r[:, qs_, oj * 512 : (oj + 1) * 512],
                        in_=out_acc[:, qs_, oj * 512 : (oj + 1) * 512],
                    )

    # Software pipeline across heads: head h+1's scores are emitted before
    # head h's out-projection, so ScalarE computes h+1's exps while the PE
    # is busy with h's out-projection matmuls.
    out_r = out.ap().rearrange("(qs p) o -> p qs o", p=128)
    fuse_store = tkp <= 1536
    sc = emit_scores(0)
    for h in range(n_heads):
        xts = emit_attnv(h, *sc)
        if h + 1 < n_heads:
            sc = emit_scores(h + 1)
        last = h == n_heads - 1
        emit_outproj(h, xts, out_r=(out_r if last and fuse_store else None))
    if not fuse_store:
        for qs_ in range(QS):
            nc.sync.dma_start(out=out_r[:, qs_, :], in_=out_acc[:, qs_, :])


def _build_kernel(nc, t, tkp, loop_n=None, n_heads=H):
    tc = tile.TileContext(nc)
    with tc, nc.allow_low_precision(
        reason="float32r matmul operands; all accumulation is fp32 in PSUM/SBUF"
    ):
        pools = (
            tc.alloc_tile_pool(name="const", bufs=1),
            tc.alloc_tile_pool(name="wpool", bufs=1),
            tc.alloc_tile_pool(name="pt", bufs=(2 * (tkp // 128) + 2 if tkp <= 1536 else tkp // 128)),
            tc.alloc_tile_pool(name="xt", bufs=9),
            tc.alloc_tile_pool(name="wo", bufs=9 if tkp <= 1536 else 5),
            tc.alloc_tile_pool(name="smalls", bufs=2),
            tc.alloc_tile_pool(name="ps_s", bufs=2, space="PSUM"),
            tc.alloc_tile_pool(name="ps_d", bufs=2, space="PSUM"),
            tc.alloc_tile_pool(name="ps_o", bufs=2, space="PSUM"),
            tc.alloc_tile_pool(name="ps_p", bufs=2, space="PSUM"),
        )
        if loop_n is None:
            _body(nc, t, pools, tkp, n_heads=n_heads)
        else:
            with tc.For_i(0, loop_n, 1):
                _body(nc, t, pools, tkp, n_heads=n_heads)
        for p in reversed(pools):
            p.release()
    nc.compile()


def _declare_io(nc, tkp):
    qT = nc.dram_tensor("qT", (D, TQC), F32R, kind="ExternalInput")
    kT = nc.dram_tensor("kT", (D, tkp), F32R, kind="ExternalInput")
    v = nc.dram_tensor("v", (tkp, DV), F32R, kind="ExternalInput")
    wqT = nc.dram_tensor("wqT", (D, R), F32R, kind="ExternalInput")
    wkT = nc.dram_tensor("wkT", (D, R), F32R, kind="ExternalInput")
    woT = nc.dram_tensor("woT", (H * DV, O), F32R, kind="ExternalInput")
    bq = nc.dram_tensor("bq", (R,), F32, kind="ExternalInput")
    bk = nc.dram_tensor("bk", (R,), F32, kind="ExternalInput")
    bo = nc.dram_tensor("bo", (O,), F32, kind="ExternalInput")
    mb = nc.dram_tensor("mb", (tkp,), F32, kind="ExternalInput")
    ones_col_d = nc.dram_tensor("ones_col_d", (128, 1), F32R, kind="ExternalInput")
    ones_row_d = nc.dram_tensor("ones_row_d", (1, 128), F32R, kind="ExternalInput")
    out = nc.dram_tensor("out", (TQC, O), F32, kind="ExternalOutput")
    return (qT, kT, v, wqT, wkT, woT, bq, bk, bo, mb, ones_col_d, ones_row_d, out)


def build_nc(tkp=1024, loop_n=None, n_heads=H):
    nc = bacc.Bacc("TRN2", target_bir_lowering=False, debug=False,
                   num_devices=NCORES)
    t = _declare_io(nc, tkp)
    _build_kernel(nc, t, tkp, loop_n=loop_n, n_heads=n_heads)
    return nc


_NC_CACHE = {}


def _get_nc(tkp):
    if tkp not in _NC_CACHE:
        _NC_CACHE[tkp] = build_nc(tkp)
    return _NC_CACHE[tkp]


def _prep_in_maps(Q, K, V, Wq, bq, Wk, bk, Wo, bo, key_padding_mask):
    Q = np.asarray(Q, dtype=np.float32)
    K = np.asarray(K, dtype=np.float32)
    V = np.asarray(V, dtype=np.float32)
    Wq = np.asarray(Wq, dtype=np.float32)
    Wk = np.asarray(Wk, dtype=np.float32)
    Wo = np.asarray(Wo, dtype=np.float32)
    bq = np.ascontiguousarray(np.asarray(bq, dtype=np.float32))
    bk = np.ascontiguousarray(np.asarray(bk, dtype=np.float32))
    bo = np.ascontiguousarray(np.asarray(bo, dtype=np.float32))
    mask = np.asarray(key_padding_mask).astype(bool)

    # key compaction: keep only unmasked keys, pad to a 128 multiple
    keep = [np.flatnonzero(~mask[b]) for b in range(B)]
    tkp = max(128, ((max(len(ix) for ix in keep) + 127) // 128) * 128)

    wqT = np.ascontiguousarray(Wq.T)
    wkT = np.ascontiguousarray(Wk.T)
    woT = np.ascontiguousarray(Wo.T)
    ones_col = np.ones((128, 1), np.float32)
    ones_row = np.ones((1, 128), np.float32)
    kT, vb, mb = [], [], []
    for b in range(B):
        ix = keep[b]
        kt = np.zeros((D, tkp), np.float32)
        kt[:, : len(ix)] = K[b].T[:, ix]
        kT.append(kt)
        vv = np.zeros((tkp, DV), np.float32)
        vv[: len(ix)] = V[b][ix]
        vb.append(vv)
        m = np.full(tkp, np.float32(MASK_BIAS), np.float32)
        m[: len(ix)] = 0.0
        mb.append(m)

    in_maps = []
    for c in range(NCORES):
        b, chunk = divmod(c, NCORES // B)
        q0 = chunk * TQC
        in_maps.append(
            {
                "qT": np.ascontiguousarray(Q[b, q0 : q0 + TQC, :].T),
                "kT": kT[b],
                "v": vb[b],
                "wqT": wqT,
                "wkT": wkT,
                "woT": woT,
                "bq": bq,
                "bk": bk,
                "bo": bo,
                "mb": mb[b],
                "ones_col_d": ones_col,
                "ones_row_d": ones_row,
            }
        )
    return tkp, in_maps


def run(inputs: dict, **spmd_kwargs):
    """Build (cached), run on 8 cores, return (full_output, BassKernelResults)."""
    tkp, in_maps = _prep_in_maps(**inputs)
    nc = _get_nc(tkp)
    res = run_bass_kernel_spmd(nc, in_maps, core_ids=list(range(NCORES)),
                               **spmd_kwargs)
    out = np.empty((B, TQ, O), np.float32)
    for c in range(NCORES):
        b, chunk = divmod(c, NCORES // B)
        q0 = chunk * TQC
        out[b, q0 : q0 + TQC, :] = res.results[c]["out"]
    return out, res


def kernel(**inputs) -> np.ndarray:
    out, _ = run(inputs)
    return out
